# revision 17
# baseline (speedup 1.0000x reference)
"""Trainium2 Bass kernel for nn_DiagonalVariance: per-dim MLPs [4->64->64->1] with softplus.

Strategy (pure data parallel over batch, 8 cores):
  - Host packs x^T = [y^T; t^T; ones] as [20, B] so all device DMAs are contiguous.
  - Per dim-pair p (2 dims), weights are packed as:
      W1p [20, 128]  (y-rows are delta-masked per dim, t-rows shared, last row = b1)
      W2p [128, 128] block-diagonal of two 64x64 blocks
      W3p [128, 32]  cols 0/1 hold W3 for the two dims, rest zero
  - softplus(z) = Ln(Exp(z) + 1): two ACT passes; the activation table set is
    pinned to natural_log_exp_and_others so it loads exactly once. The kernel
    is bound by ScalarE throughput (1 elem/cycle/lane for every ACT func):
    ~2 passes over 33.5M hidden elements per core. b2/b3 are fused into the
    Exp pass via per-partition bias APs; b1 rides a ones-row of x^T.
  - E=exp(z) is stored as fp16 (rounding E perturbs softplus by <= relerr(E),
    and |z|<=~8 here so no overflow); Ln ops are merged across pair groups
    (ln_group) to amortize the ~550-cycle per-op ACT overhead.
  - Matmuls run as float32r (full-rate PE mode on fp32 data; plain fp32 is
    4 cycles/row). L3 accumulates all 8 pairs into one psum tile whose rows
    0..15 are the 16 output dims.
  - Output is written as [16, BC] per core and transposed on the host.
"""

import numpy as np
from contextlib import ExitStack, nullcontext

import concourse.bass as bass
import concourse.bacc as bacc
import concourse.tile as tile
from concourse import mybir
from concourse.hw_specs import get_activation_tables

F = mybir.ActivationFunctionType
FP32 = mybir.dt.float32
FP32R = mybir.dt.float32r
FP16 = mybir.dt.float16

B = 262144
D = 16
TE = 3
H = 64
NCORES = 8
BC = B // NCORES          # 32768 batch points per core
NB = 2048                 # batch tile per pair-step
NMM = 512                 # max fp32 moving free dim per matmul
NPAIR = D // 2            # 8 dim-pairs
NTILES = BC // NB

_ACT_SET = "natural_log_exp_and_others"


def _pin_act_tables(arch):
    """Restrict Exp/Ln to one table set so bacc emits a single table load."""
    tables = get_activation_tables(arch)
    for name, funcs in tables.items():
        if name != _ACT_SET:
            funcs.discard(F.Exp)
            funcs.discard(F.Ln)


def build(ntiles=NTILES, reps=1, mm_dtype=FP32R, fp16_e=True, nb=None, dve_copy=False, z_bufs=1, ln_group=2):
    nc = bacc.Bacc("TRN2", target_bir_lowering=False, debug=False,
                   enable_asserts=True, num_devices=NCORES)
    _pin_act_tables(nc.m.arch)
    NB = nb or globals()["NB"]
    E_DT = FP16 if fp16_e else mm_dtype

    xT = nc.dram_tensor("xT", [20, BC], mm_dtype, kind="ExternalInput")
    w1 = nc.dram_tensor("w1", [20, NPAIR * 128], mm_dtype, kind="ExternalInput")
    w2 = nc.dram_tensor("w2", [128, NPAIR * 128], mm_dtype, kind="ExternalInput")
    w3 = nc.dram_tensor("w3", [128, NPAIR * 128], mm_dtype, kind="ExternalInput")
    b2 = nc.dram_tensor("b2", [128, NPAIR], FP32, kind="ExternalInput")
    b3 = nc.dram_tensor("b3", [128, 1], FP32, kind="ExternalInput")
    # output row d holds dim d, contiguous batch columns
    out = nc.dram_tensor("out", [D, ntiles * NB], FP32, kind="ExternalOutput")

    def mm(out_ap, lhsT, rhs, **kw):
        nc.tensor.matmul(out_ap, lhsT, rhs, **kw)

    with tile.TileContext(nc) as tc:
        with ExitStack() as ctx:
            wpool = ctx.enter_context(tc.tile_pool(name="w", bufs=1))
            xpool = ctx.enter_context(tc.tile_pool(name="x", bufs=2))
            hpool1 = ctx.enter_context(tc.tile_pool(name="h1", bufs=2))
            hpool2 = ctx.enter_context(tc.tile_pool(name="h2", bufs=NPAIR // ln_group + 1))
            opool = ctx.enter_context(tc.tile_pool(name="o", bufs=2))
            epool = ctx.enter_context(tc.tile_pool(name="e", bufs=3))
            zpool1 = ctx.enter_context(tc.tile_pool(name="z1", bufs=z_bufs, space="PSUM"))
            zpool2 = ctx.enter_context(tc.tile_pool(name="z2", bufs=z_bufs, space="PSUM"))
            zpool3 = zpool1

            w1sb = wpool.tile([20, NPAIR * 128], mm_dtype)
            w2sb = wpool.tile([128, NPAIR * 128], mm_dtype)
            w3sb = wpool.tile([128, NPAIR * 128], mm_dtype)
            b2sb = wpool.tile([128, NPAIR], FP32)
            b3sb = wpool.tile([128, 1], FP32)
            nc.sync.dma_start(out=w1sb, in_=w1[:, :])
            nc.sync.dma_start(out=w2sb, in_=w2[:, :])
            nc.sync.dma_start(out=w3sb, in_=w3[:, :])
            nc.sync.dma_start(out=b2sb, in_=b2[:, :])
            nc.sync.dma_start(out=b3sb, in_=b3[:, :])

            loop_cm = tc.For_i(0, reps, 1) if reps > 1 else nullcontext()
            with loop_cm:
                for i in range(ntiles):
                    xt = xpool.tile([20, NB], mm_dtype)
                    nc.sync.dma_start(out=xt, in_=xT[:, i * NB:(i + 1) * NB])

                    G = ln_group
                    h2s = []
                    for g in range(NPAIR // G):
                        pg = range(g * G, (g + 1) * G)
                        e1g = epool.tile([128, G, NB], E_DT, tag="e")
                        for j, p in enumerate(pg):
                            z1 = zpool1.tile([128, NB], FP32, tag="z1")
                            for m in range(NB // NMM):
                                s = slice(m * NMM, (m + 1) * NMM)
                                mm(z1[:, s], w1sb[:, p * 128:(p + 1) * 128], xt[:, s],
                                   start=True, stop=True)
                            nc.scalar.activation(e1g[:, j, :], z1, F.Exp)
                        h1g = hpool1.tile([128, G, NB], mm_dtype)
                        nc.scalar.activation(h1g, e1g, F.Ln, bias=1.0)

                        e2g = epool.tile([128, G, NB], E_DT, tag="e")
                        for j, p in enumerate(pg):
                            z2 = zpool2.tile([128, NB], FP32)
                            for m in range(NB // NMM):
                                s = slice(m * NMM, (m + 1) * NMM)
                                mm(z2[:, s], w2sb[:, p * 128:(p + 1) * 128],
                                   h1g[:, j, s], start=True, stop=True)
                            nc.scalar.activation(e2g[:, j, :], z2, F.Exp,
                                                 bias=b2sb[:, p:p + 1])
                        h2g = hpool2.tile([128, G, NB], mm_dtype)
                        nc.scalar.activation(h2g, e2g, F.Ln, bias=1.0)
                        for j, p in enumerate(pg):
                            h2s.append(h2g[:, j, :])

                    # all 8 pairs accumulate into one [128, NB] psum tile;
                    # pair p's lhsT has its W3 columns at 2p/2p+1, so rows
                    # 0..15 collect all dims and rows 16..127 stay zero.
                    z3 = zpool3.tile([128, NB], FP32, tag="z1")
                    for m in range(NB // NMM):
                        s = slice(m * NMM, (m + 1) * NMM)
                        for p in range(NPAIR):
                            mm(z3[:, s], w3sb[:, p * 128:(p + 1) * 128],
                               h2s[p][:, s], start=(p == 0), stop=(p == NPAIR - 1))
                    o3 = opool.tile([D, NB], FP32)
                    if fp16_e:
                        e3 = epool.tile([128, NB], E_DT, tag="e")
                        nc.scalar.activation(e3[:D, :], z3[:D, :], F.Exp, bias=b3sb[:D, :])
                        nc.scalar.activation(o3, e3[:D, :], F.Ln, bias=1.0)
                    else:
                        nc.scalar.activation(o3, z3[:D, :], F.Exp, bias=b3sb[:D, :])
                        nc.scalar.activation(o3, o3, F.Ln, bias=1.0)
                    nc.sync.dma_start(out=out[:, i * NB:(i + 1) * NB], in_=o3)
    nc.compile()
    return nc


def _pack_inputs(t, y, W1, b1, W2, b2, W3, b3):
    """Host-side packing. Returns per-core input maps."""
    t = np.asarray(t, np.float32)
    y = np.asarray(y, np.float32)
    W1 = np.asarray(W1, np.float32)
    b1 = np.asarray(b1, np.float32)
    W2 = np.asarray(W2, np.float32)
    b2 = np.asarray(b2, np.float32)
    W3 = np.asarray(W3, np.float32)
    b3 = np.asarray(b3, np.float32)

    xT = np.empty((20, B), np.float32)
    xT[:D] = y.T
    xT[D:D + TE] = t.T
    xT[D + TE] = 1.0

    w1p = np.zeros((20, NPAIR * 128), np.float32)
    w2p = np.zeros((128, NPAIR * 128), np.float32)
    w3p = np.zeros((128, NPAIR * 128), np.float32)
    b2p = np.zeros((128, NPAIR), np.float32)
    b3p = np.zeros((128, 1), np.float32)
    for p in range(NPAIR):
        for a in range(2):
            d = 2 * p + a
            c = slice(p * 128 + 64 * a, p * 128 + 64 * a + 64)
            w1p[d, c] = W1[d, 0, :]
            w1p[D:D + TE, c] = W1[d, 1:1 + TE, :]
            w1p[D + TE, c] = b1[d, :]
            w2p[64 * a:64 * a + 64, p * 128 + 64 * a:p * 128 + 64 * a + 64] = W2[d]
            w3p[64 * a:64 * a + 64, p * 128 + d] = W3[d, :, 0]
            b2p[64 * a:64 * a + 64, p] = b2[d]
            b3p[d, 0] = b3[d, 0]

    in_maps = []
    for c in range(NCORES):
        in_maps.append({
            "xT": np.ascontiguousarray(xT[:, c * BC:(c + 1) * BC]),
            "w1": w1p, "w2": w2p, "w3": w3p, "b2": b2p, "b3": b3p,
        })
    return in_maps


def _unpack_output(results):
    return np.concatenate([results[c]["out"].T for c in range(NCORES)], axis=0)


def make_runner(nc):
    """Build a reusable jitted SPMD callable for `nc` (axon PJRT path)."""
    import jax
    from jax.sharding import Mesh, PartitionSpec, NamedSharding
    from jax.experimental.shard_map import shard_map
    from concourse import bass2jax

    bass2jax.install_neuronx_cc_hook()
    partition_name = nc.partition_id_tensor.name if nc.partition_id_tensor else None
    in_names, out_names, out_avals = [], [], []
    for alloc in nc.m.functions[0].allocations:
        if not isinstance(alloc, mybir.MemoryLocationSet):
            continue
        name = alloc.memorylocations[0].name
        if alloc.kind == "ExternalInput":
            if name != partition_name:
                in_names.append(name)
        elif alloc.kind == "ExternalOutput":
            out_names.append(name)
            out_avals.append(jax.core.ShapedArray(tuple(alloc.tensor_shape),
                                                  mybir.dt.np(alloc.dtype)))
    all_in = in_names + out_names + ([partition_name] if partition_name else [])

    def _body(*args):
        operands = list(args)
        if partition_name is not None:
            operands.append(bass2jax.partition_id_tensor())
        outs = bass2jax._bass_exec_p.bind(
            *operands, out_avals=tuple(out_avals),
            in_names=tuple(all_in), out_names=tuple(out_names),
            lowering_input_output_aliases=(), sim_require_finite=True,
            sim_require_nnan=True, nc=nc)
        return tuple(outs)

    mesh = Mesh(np.asarray(jax.devices()[:NCORES]), ("core",))
    n = len(in_names) + len(out_names)
    sharded = jax.jit(shard_map(_body, mesh=mesh,
                                in_specs=(PartitionSpec("core"),) * n,
                                out_specs=(PartitionSpec("core"),) * len(out_names),
                                check_rep=False), keep_unused=True)
    shard0 = NamedSharding(mesh, PartitionSpec("core"))
    zeros = [jax.device_put(np.zeros((NCORES * a.shape[0], *a.shape[1:]), a.dtype),
                            shard0) for a in out_avals]

    def stage(in_maps):
        return [jax.device_put(
            np.concatenate([np.asarray(in_maps[c][nm]) for c in range(NCORES)], axis=0),
            shard0) for nm in in_names]

    def run_staged(dev_in):
        out_arrs = sharded(*dev_in, *zeros)
        jax.block_until_ready(out_arrs)
        return out_arrs

    def run(in_maps):
        out_arrs = run_staged(stage(in_maps))
        return [
            {name: np.asarray(out_arrs[i]).reshape(NCORES, *out_avals[i].shape)[c]
             for i, name in enumerate(out_names)}
            for c in range(NCORES)
        ]

    run.stage = stage
    run.run_staged = run_staged
    run.out_names = out_names
    run.out_avals = out_avals
    return run


_CACHED = {}


def _get_runner():
    if "runner" not in _CACHED:
        _CACHED["runner"] = make_runner(build())
    return _CACHED["runner"]


def kernel(t, y, W1, b1, W2, b2, W3, b3):
    run = _get_runner()
    in_maps = _pack_inputs(t, y, W1, b1, W2, b2, W3, b3)
    results = run(in_maps)
    return _unpack_output(results)
